# revision 19
# baseline (speedup 1.0000x reference)
"""DBToAmplitude kernel for Trainium2: out = 10 ** features, elementwise.

features: (64, 80, 20000) float32, values in [0, 1).  Harness tolerance is
rel_err < 2e-2, so the kernel runs in reduced precision: the host quantizes
x to uint8 (q = rint(255 x); grid error <= 0.5/255 -> rel error on 10**x
<= 4.5e-3), each core streams its (5, 128, 20000) uint8 shard HBM->SBUF in
2.56 MB tiles, and one ScalarE activation pass per tile computes
Exp(ln(10)/255 * q) — the engine's affine prescale dequantizes for free,
LUT spline error ~1.1e-5 — writing float16 (rel error 2**-12).  The fp16
result streams back and the host upcasts to fp32 exactly.  Measured HW
rel error 5.0e-3, 4x inside the gate.

Memory-bound: 38.4 MB HBM traffic per core (uint8 in + fp16 out, vs
102.4 MB for the fp32 kernel).  The three DMA-capable DGE paths carry equal
bytes per sweep: loads (12.8 MB) on sync/HWDGE, each fp16 store half
(12.8 MB) on gpsimd/SWDGE and scalar/HWDGE ("sp2" split).  Big tiles beat
small ones (measured 635 GB/s/core combined at f=20000+sp2 vs 447 at
f=5000 two-queue), and the ACT pass hides entirely under the DMA stream
(a no-ACT echo probe measured the same-shape DMA floor at 615 GB/s/core).
"""

import math
import time

import numpy as np

import concourse.bacc as bacc
import concourse.bass as bass
import concourse.mybir as mybir
import concourse.tile as tile
from concourse.bass_utils import run_bass_kernel_spmd

N_CORES = 8
SHAPE = (64, 80, 20000)
TOTAL = SHAPE[0] * SHAPE[1] * SHAPE[2]          # 102,400,000
PER_CORE = TOTAL // N_CORES                     # 12,800,000
P = 128
FREE = PER_CORE // P                            # 100,000
F = 20000                                       # free-dim elements per tile
N_TILES = FREE // F                             # 5 tiles/core
LN10 = math.log(10.0)

VARIANT = "q8_sp2"

_NC_CACHE = {}


def build_nc(variant=VARIANT, n_sweeps=1, f=F, bufs=(2, 2), pool_mode="stack"):
    n_tiles = FREE // f
    assert n_tiles * f == FREE
    q8 = variant.startswith("q8")
    echo = variant.startswith("echo")  # DMA-ceiling probe: no ACT, y=x bytes
    in_dt = mybir.dt.uint8 if (q8 or echo) else mybir.dt.float16
    dt = mybir.dt.uint8 if echo else mybir.dt.float16
    # q8: host sends round(x*255); ACT's affine prescale dequantizes for free.
    act_scale = LN10 / 255.0 if q8 else LN10
    nc = bacc.Bacc("TRN2", target_bir_lowering=False, debug=False)
    x = nc.dram_tensor("x", [n_tiles, P, f], in_dt, kind="ExternalInput")
    y = nc.dram_tensor("y", [n_tiles, P, f], dt, kind="ExternalOutput")
    xap, yap = x.ap(), y.ap()
    with tile.TileContext(nc, pool_alloc_mode=pool_mode) as tc:
        with (
            tc.tile_pool(name="pin", bufs=bufs[0]) as pin,
            tc.tile_pool(name="pout", bufs=bufs[1]) as pout,
        ):
            for _ in range(n_sweeps):
                for i in range(n_tiles):
                    if echo:
                        tin = pin.tile([P, f], in_dt)
                        nc.sync.dma_start(tin[:], xap[i][:])
                        nc.gpsimd.dma_start(yap[i][:], tin[:])
                        continue
                    base = variant[3:] if q8 else variant
                    if base == "h2q":
                        load_eng, store_eng = nc.sync, nc.gpsimd
                    elif base == "sp2":
                        load_eng, store_eng = nc.sync, None
                    elif base == "swp":
                        # HWDGE for the (2x bigger) store stream
                        load_eng, store_eng = nc.gpsimd, nc.sync
                    elif base == "mix":
                        load_eng = nc.sync
                        store_eng = nc.gpsimd if i % 2 == 0 else nc.scalar
                    elif base == "hw2":
                        # both HWDGE rings: loads on SP, stores on ACT
                        load_eng, store_eng = nc.sync, nc.scalar
                    elif base == "h4q":
                        load_eng = nc.sync if i % 2 == 0 else nc.vector
                        store_eng = nc.gpsimd if i % 2 == 0 else nc.tensor
                    elif base == "h4q_swap":
                        load_eng = nc.sync if i % 2 == 0 else nc.tensor
                        store_eng = nc.gpsimd if i % 2 == 0 else nc.vector
                    else:
                        raise ValueError(variant)
                    tin = pin.tile([P, f], in_dt)
                    load_eng.dma_start(tin[:], xap[i][:])
                    ty = pout.tile([P, f], dt)
                    nc.scalar.activation(
                        ty[:], tin[:], mybir.ActivationFunctionType.Exp,
                        scale=act_scale,
                    )
                    if base == "sp2":
                        # split each store across both DGE paths concurrently
                        half = f // 2
                        nc.gpsimd.dma_start(yap[i][:, :half], ty[:, :half])
                        nc.scalar.dma_start(yap[i][:, half:], ty[:, half:])
                    else:
                        store_eng.dma_start(yap[i][:], ty[:])
    nc.compile()
    return nc


def _get_nc():
    if _NC_CACHE.get("variant") != VARIANT:
        _NC_CACHE.clear()
        _NC_CACHE["nc"] = build_nc(VARIANT)
        _NC_CACHE["variant"] = VARIANT
    return _NC_CACHE["nc"]


def kernel(features: np.ndarray) -> np.ndarray:
    feats = np.asarray(features, dtype=np.float32)
    if VARIANT.startswith("q8"):
        feats = np.rint(feats * np.float32(255.0))
        np.clip(feats, 0.0, 255.0, out=feats)
        feats = feats.astype(np.uint8)
    else:
        feats = feats.astype(np.float16)
    shards = feats.reshape(N_CORES, N_TILES, P, F)
    in_maps = [{"x": shards[c]} for c in range(N_CORES)]
    last_err = None
    for attempt in range(4):
        try:
            res = run_bass_kernel_spmd(
                _get_nc(), in_maps, core_ids=list(range(N_CORES))
            )
            break
        except Exception as e:  # transient NRT_EXEC_UNIT_UNRECOVERABLE etc.
            last_err = e
            _NC_CACHE.clear()
            time.sleep(10 * (attempt + 1))
            try:
                import jax
                from jax.extend import backend as _jex_backend

                jax.clear_caches()
                _jex_backend.clear_backends()
            except Exception:
                pass
    else:
        raise last_err
    out = np.stack([res.results[c]["y"] for c in range(N_CORES)])
    return out.reshape(SHAPE).astype(np.float32)
